# revision 12
# baseline (speedup 1.0000x reference)
"""Trainium2 Bass kernel for nn_Chebnet (3x ChebConv + BN + per-node FC head).

Strategy: data-parallel over batch B=32 across 8 cores (4 per core).
Host precomputes the dense normalized propagation matrix P = T_1 and the
Chebyshev polynomial matrices A_k = T_k(P) (shared by all conv layers),
so all per-edge gather/scatter becomes dense matmuls on the PE.

Layouts per core:
  "normal"  h_N  [128p(n), nt=8, bc=256]   (node on partitions)
  "flipped" U_kF [128p(bc), ch=2, n=1024]  (batch*channel on partitions)
Props produce flipped U_k from normal h via lhsT=h chunks, rhs=A_k^T.
The channel-mix einsum consumes flipped U_k with block-diagonal W (2
batches at once) and lands the layer output back in normal layout,
accumulating over k directly in persistent PSUM banks (so only one U_k
is alive at a time). All matmuls run in float32r (bitcast of f32 —
full-rate on the PE for moving dim >= 256, near-fp32 precision).
BatchNorm stats need full-batch sums -> one tiny [128,32] f32 AllReduce
per BN layer (4 total).
"""

import numpy as np

N_CORES = 8
B, N, C, E = 32, 1024, 64, 32768
B_LOC = B // N_CORES          # 4 batch elements per core
NT = N // 128                 # 8 node tiles
BC = B_LOC * C                # 256
EPS = 1e-5

_CACHE = {}


# ---------------------------------------------------------------- device code

def _build_nc():
    import concourse.bacc as bacc
    import concourse.mybir as mybir
    import concourse.tile as tile

    F32 = mybir.dt.float32
    F32R = mybir.dt.float32r
    AF = mybir.ActivationFunctionType

    def r32(ap):
        return ap.bitcast(F32R)

    nc = bacc.Bacc("TRN2", target_bir_lowering=False, debug=False,
                   enable_asserts=False, num_devices=N_CORES)

    d_xn = nc.dram_tensor("xn", [128, NT, BC], F32R, kind="ExternalInput")
    d_a = [nc.dram_tensor(f"a{k}t", [128, NT, N], F32R, kind="ExternalInput")
           for k in (1, 2, 3, 4)]
    d_wblk1 = nc.dram_tensor("wblk1", [128, 5 * 128], F32R, kind="ExternalInput")
    d_wblk2 = nc.dram_tensor("wblk2", [128, 5 * 128], F32R, kind="ExternalInput")
    d_wblk3 = nc.dram_tensor("wblk3", [128, 3 * 2], F32R, kind="ExternalInput")
    d_brow1 = nc.dram_tensor("brow1", [1, 128], F32R, kind="ExternalInput")
    d_brow2 = nc.dram_tensor("brow2", [1, 128], F32R, kind="ExternalInput")
    d_brow3 = nc.dram_tensor("brow3", [1, 2], F32R, kind="ExternalInput")
    d_ones = nc.dram_tensor("onesrow", [1, 128], F32R, kind="ExternalInput")
    d_ident = nc.dram_tensor("ident", [128, 128], F32R, kind="ExternalInput")
    d_bng = nc.dram_tensor("bng", [128, 4, NT], F32, kind="ExternalInput")
    d_bnb = nc.dram_tensor("bnb", [128, 4, NT], F32, kind="ExternalInput")
    d_fw1 = nc.dram_tensor("fw1", [128, NT, 16], F32, kind="ExternalInput")
    d_fb1 = nc.dram_tensor("fb1", [128, NT, 16], F32, kind="ExternalInput")
    d_fw2 = nc.dram_tensor("fw2", [128, NT, 16], F32, kind="ExternalInput")
    d_fb2 = nc.dram_tensor("fb2", [128, NT], F32, kind="ExternalInput")
    d_y = nc.dram_tensor("y", [B_LOC, N], F32, kind="ExternalOutput")

    cc_in = [nc.dram_tensor(f"ccin{i}", [128, 32], F32) for i in range(4)]
    cc_out = [nc.dram_tensor(f"ccout{i}", [128, 32], F32, addr_space="Shared")
              for i in range(4)]
    groups = [list(range(N_CORES))]

    with tile.TileContext(nc) as tc:
        with (
            tc.tile_pool(name="const", bufs=1) as cp,
            tc.tile_pool(name="work", bufs=1) as wp,
            tc.tile_pool(name="upool", bufs=2) as up,
            tc.tile_pool(name="pprop", bufs=2, space="PSUM") as pp,
            tc.tile_pool(name="peins", bufs=4, space="PSUM") as pe,
            tc.tile_pool(name="ptr", bufs=2, space="PSUM") as pt,
        ):
            # ---- persistent loads
            def load(dram, shape, dt, tag):
                t = cp.tile(shape, dt, tag=tag)
                nc.sync.dma_start(t[:], dram[:])
                return t

            t_xn = load(d_xn, [128, NT, BC], F32R, "xn")
            t_a = [load(d_a[i], [128, NT, N], F32R, f"a{i}") for i in range(4)]
            t_w1 = load(d_wblk1, [128, 5 * 128], F32R, "w1")
            t_w2 = load(d_wblk2, [128, 5 * 128], F32R, "w2")
            t_w3 = load(d_wblk3, [128, 6], F32R, "w3")
            t_br1 = load(d_brow1, [1, 128], F32R, "br1")
            t_br2 = load(d_brow2, [1, 128], F32R, "br2")
            t_br3 = load(d_brow3, [1, 2], F32R, "br3")
            t_ones = load(d_ones, [1, 128], F32R, "ones")
            t_id = load(d_ident, [128, 128], F32R, "ident")
            t_bng = load(d_bng, [128, 4, NT], F32, "bng")
            t_bnb = load(d_bnb, [128, 4, NT], F32, "bnb")
            t_fw1 = load(d_fw1, [128, NT, 16], F32, "fw1")
            t_fb1 = load(d_fb1, [128, NT, 16], F32, "fb1")
            t_fw2 = load(d_fw2, [128, NT, 16], F32, "fw2")
            t_fb2 = load(d_fb2, [128, NT], F32, "fb2")

            junk = wp.tile([128, 256], F32, tag="junk")
            eps_t = wp.tile([128, 1], F32, tag="eps")
            nc.vector.memset(eps_t[:], EPS)

            def make_U0(h_N):
                """16 PE transposes: normal [n, bc] -> flipped [bc, n]."""
                U0 = up.tile([128, 2, N], F32R, tag="U")
                for jt in range(NT):
                    for ch in range(2):
                        ps = pt.tile([128, 128], F32R, tag="tr")
                        nc.tensor.transpose(
                            ps[:], h_N[:, jt, ch * 128:(ch + 1) * 128], t_id[:])
                        nc.vector.tensor_copy(
                            U0[:, ch, jt * 128:(jt + 1) * 128], ps[:])
                return U0

            def bn_from_stats(lidx, stats, divisor, halves):
                """AllReduce stats -> per-node scale a, shift d ([128, NT])."""
                nc.sync.dma_start(cc_in[lidx][:], stats[:])
                nc.gpsimd.collective_compute(
                    "AllReduce", mybir.AluOpType.add, replica_groups=groups,
                    ins=[cc_in[lidx][:]], outs=[cc_out[lidx][:]])
                g = wp.tile([128, 32], F32, tag="gstats")
                nc.sync.dma_start(g[:], cc_out[lidx][:])
                s1 = wp.tile([128, NT], F32, tag="s1")
                s2 = wp.tile([128, NT], F32, tag="s2")
                if halves:
                    nc.vector.tensor_add(s1[:], g[:, 0:8], g[:, 8:16])
                    nc.vector.tensor_add(s2[:], g[:, 16:24], g[:, 24:32])
                else:
                    nc.vector.tensor_copy(s1[:], g[:, 0:8])
                    nc.vector.tensor_copy(s2[:], g[:, 16:24])
                mean = wp.tile([128, NT], F32, tag="mean")
                var = wp.tile([128, NT], F32, tag="var")
                nc.vector.tensor_scalar_mul(mean[:], s1[:], 1.0 / divisor)
                nc.vector.tensor_scalar_mul(var[:], s2[:], 1.0 / divisor)
                msq = wp.tile([128, NT], F32, tag="msq")
                nc.vector.tensor_mul(msq[:], mean[:], mean[:])
                nc.vector.tensor_sub(var[:], var[:], msq[:])
                std = wp.tile([128, NT], F32, tag="std")
                nc.scalar.activation(std[:], var[:], AF.Sqrt, bias=eps_t[:])
                inv = wp.tile([128, NT], F32, tag="inv")
                nc.vector.reciprocal(inv[:], std[:])
                a = wp.tile([128, NT], F32, tag="bna")
                d = wp.tile([128, NT], F32, tag="bnd")
                nc.vector.tensor_mul(a[:], inv[:], t_bng[:, lidx, :])
                nc.vector.tensor_mul(d[:], mean[:], a[:])
                nc.vector.tensor_sub(d[:], t_bnb[:, lidx, :], d[:])
                return a, d

            def cheb_layer(lidx, K, h_N, t_wblk, t_brow, ncol, divisor,
                           out_dt, out_tag):
                """One ChebConv + bias + relu + BN layer.

                Einsum accumulates over k in persistent PSUM banks while
                U_{k+1} is produced, so only one U_k tile is alive.
                ncol = 2 * C_out per (t, ch) region.
                """
                # SBUF accumulator for the einsum (PSUM groups cannot be
                # interleaved across regions of a bank), DVE accumulates.
                acc = wp.tile([128, NT, 2 * ncol], F32, tag="acc")

                def einsum_k(k, Uk):
                    for t in range(NT):
                        for ch in range(2):
                            ps = pe.tile([128, ncol], F32, tag="eins",
                                         name="eins")
                            nc.tensor.matmul(
                                ps[:],
                                Uk[:, ch, t * 128:(t + 1) * 128],
                                t_wblk[:, k * ncol:(k + 1) * ncol],
                                start=True, stop=(k != 0))
                            reg = acc[:, t, ch * ncol:(ch + 1) * ncol]
                            if k == 0:
                                # fold the (rank-1) bias row into group 0
                                nc.tensor.matmul(ps[:], t_ones[:1, :],
                                                 t_brow[:1, :],
                                                 start=False, stop=True)
                                nc.vector.tensor_copy(reg, ps[:])
                            else:
                                nc.vector.tensor_add(reg, reg, ps[:])

                U = make_U0(h_N)
                einsum_k(0, U)
                for k in range(1, K):
                    Uk = up.tile([128, 2, N], F32R, tag="U")
                    for ch in range(2):
                        for hf in range(2):
                            ps = pp.tile([128, 512], F32, tag="prop")
                            for jt in range(NT):
                                nc.tensor.matmul(
                                    ps[:],
                                    h_N[:, jt, ch * 128:(ch + 1) * 128],
                                    t_a[k - 1][:, jt, hf * 512:(hf + 1) * 512],
                                    start=(jt == 0), stop=(jt == NT - 1))
                            nc.vector.tensor_copy(
                                Uk[:, ch, hf * 512:(hf + 1) * 512], ps[:])
                    einsum_k(k, Uk)

                # relu + stats
                stats = wp.tile([128, 32], F32, tag="stats")
                rN = wp.tile([128, NT, 256], F32, tag="rN")
                for t in range(NT):
                    for ch in range(2):
                        c0 = ch * 8 + t
                        nc.scalar.activation(
                            rN[:, t, ch * ncol:(ch + 1) * ncol],
                            acc[:, t, ch * ncol:(ch + 1) * ncol],
                            AF.Relu, accum_out=stats[:, c0:c0 + 1])
                        nc.scalar.activation(
                            junk[:, 0:ncol],
                            rN[:, t, ch * ncol:(ch + 1) * ncol],
                            AF.Square, accum_out=stats[:, 16 + c0:17 + c0])

                a, d = bn_from_stats(lidx, stats, divisor, halves=True)
                hout = wp.tile([128, NT, 2 * ncol], out_dt, tag=out_tag)
                for t in range(NT):
                    nc.scalar.activation(hout[:, t, 0:2 * ncol],
                                         rN[:, t, 0:2 * ncol], AF.Identity,
                                         bias=d[:, t:t + 1], scale=a[:, t:t + 1])
                return hout

            # ---- layers 1..3
            h1 = cheb_layer(0, 5, t_xn, t_w1, t_br1, 128, float(B * C),
                            F32R, "h")
            h2 = cheb_layer(1, 5, h1, t_w2, t_br2, 128, float(B * C),
                            F32R, "h")
            h3 = cheb_layer(2, 3, h2, t_w3, t_br3, 2, float(B),
                            F32, "h3")  # [128, NT, 4] f32

            # ---- fc1 (per-node 1->16) + relu + bn4
            h4p = wp.tile([128, NT, 4 * 16], F32, tag="h4p")
            stats4 = wp.tile([128, 32], F32, tag="stats")
            r4 = wp.tile([128, NT, 64], F32, tag="r4")
            for t in range(NT):
                for b in range(B_LOC):
                    sl = h4p[:, t, b * 16:(b + 1) * 16]
                    nc.vector.tensor_scalar_mul(sl, t_fw1[:, t, :],
                                                h3[:, t, b:b + 1])
                    nc.vector.tensor_add(sl, sl, t_fb1[:, t, :])
                nc.scalar.activation(r4[:, t, :], h4p[:, t, :], AF.Relu,
                                     accum_out=stats4[:, t:t + 1])
                nc.scalar.activation(junk[:, 0:64], r4[:, t, :], AF.Square,
                                     accum_out=stats4[:, 16 + t:17 + t])
            a4, d4 = bn_from_stats(3, stats4, float(B * 16), halves=False)
            h4 = wp.tile([128, NT, 64], F32, tag="h4")
            for t in range(NT):
                nc.scalar.activation(h4[:, t, :], r4[:, t, :], AF.Identity,
                                     bias=d4[:, t:t + 1], scale=a4[:, t:t + 1])

            # ---- fc2 (per-node 16->1) + output
            y_r = d_y[:].rearrange("b (t p) -> t p b", p=128)
            tmp2 = wp.tile([128, 16], F32, tag="tmp2")
            for t in range(NT):
                res = wp.tile([128, B_LOC], F32, tag="res")
                for b in range(B_LOC):
                    nc.vector.tensor_mul(tmp2[:], h4[:, t, b * 16:(b + 1) * 16],
                                         t_fw2[:, t, :])
                    nc.scalar.activation(junk[:, 0:16], tmp2[:], AF.Identity,
                                         accum_out=res[:, b:b + 1])
                nc.vector.tensor_scalar_add(res[:], res[:], t_fb2[:, t:t + 1])
                nc.sync.dma_start(y_r[t], res[:])

    nc.compile()
    return nc


# ---------------------------------------------------------------- host side

def _prep_consts(edge_index, cheb_w1, cheb_b1, cheb_w2, cheb_b2, cheb_w3,
                 cheb_b3, bn_g1, bn_b1, bn_g2, bn_b2, bn_g3, bn_b3, bn_g4,
                 bn_b4, fc_w1, fc_b1, fc_w2, fc_b2):
    f32 = np.float32
    src = np.asarray(edge_index[0], dtype=np.int64)
    tgt = np.asarray(edge_index[1], dtype=np.int64)
    deg = np.bincount(src, minlength=N).astype(np.float64)
    dis = np.where(deg > 0, 1.0 / np.sqrt(np.where(deg > 0, deg, 1.0)), 0.0)
    norm = -dis[src] * dis[tgt]
    P = np.zeros((N, N), dtype=np.float64)
    np.add.at(P, (tgt, src), norm)

    A = [P]                                     # A_1
    A.append(2.0 * P @ A[0] - np.eye(N))        # A_2
    A.append(2.0 * P @ A[1] - A[0])             # A_3
    A.append(2.0 * P @ A[2] - A[1])             # A_4

    def a_layout(Ak):
        # SBUF [128p(j), jt, i] with A^T[j, i] = A[i, j]
        return np.ascontiguousarray(
            Ak.T.reshape(NT, 128, N).transpose(1, 0, 2).astype(f32))

    def wblk(Wl, K, cout):
        out = np.zeros((128, K * 2 * cout), dtype=f32)
        for k in range(K):
            blk = out[:, k * 2 * cout:(k + 1) * 2 * cout]
            blk[0:64, 0:cout] = Wl[k]
            blk[64:128, cout:2 * cout] = Wl[k]
        return out

    def pernode(v):                             # [N, ...] -> [128, NT, ...]
        v = np.asarray(v, dtype=f32)
        return np.ascontiguousarray(
            v.reshape(NT, 128, *v.shape[1:]).transpose(
                1, 0, *range(2, v.ndim + 1)))

    consts = {
        "a1t": a_layout(A[0]), "a2t": a_layout(A[1]),
        "a3t": a_layout(A[2]), "a4t": a_layout(A[3]),
        "wblk1": wblk(np.asarray(cheb_w1, f32), 5, 64),
        "wblk2": wblk(np.asarray(cheb_w2, f32), 5, 64),
        "wblk3": wblk(np.asarray(cheb_w3, f32), 3, 1),
        "brow1": np.tile(np.asarray(cheb_b1, f32), 2)[None],
        "brow2": np.tile(np.asarray(cheb_b2, f32), 2)[None],
        "brow3": np.tile(np.asarray(cheb_b3, f32), 2)[None],
        "onesrow": np.ones((1, 128), dtype=f32),
        "ident": np.eye(128, dtype=f32),
        "bng": np.ascontiguousarray(np.stack(
            [pernode(g) for g in (bn_g1, bn_g2, bn_g3, bn_g4)], axis=1)),
        "bnb": np.ascontiguousarray(np.stack(
            [pernode(b) for b in (bn_b1, bn_b2, bn_b3, bn_b4)], axis=1)),
        "fw1": pernode(np.asarray(fc_w1, f32)[:, 0, :]),
        "fb1": pernode(np.asarray(fc_b1, f32)),
        "fw2": pernode(np.asarray(fc_w2, f32)[:, :, 0]),
        "fb2": pernode(np.asarray(fc_b2, f32)[:, 0]),
    }
    return consts


def _shard_x(x):
    x = np.asarray(x, dtype=np.float32)
    shards = []
    for c in range(N_CORES):
        xb = x[c * B_LOC:(c + 1) * B_LOC]                     # [4, N, C]
        xn = np.ascontiguousarray(
            xb.transpose(1, 0, 2).reshape(N, BC)
              .reshape(NT, 128, BC).transpose(1, 0, 2))
        shards.append(xn)
    return shards


def get_nc():
    if "nc" not in _CACHE:
        _CACHE["nc"] = _build_nc()
    return _CACHE["nc"]


def make_in_maps(inputs):
    consts = _prep_consts(
        inputs["edge_index"], inputs["cheb_w1"], inputs["cheb_b1"],
        inputs["cheb_w2"], inputs["cheb_b2"], inputs["cheb_w3"],
        inputs["cheb_b3"], inputs["bn_g1"], inputs["bn_b1"], inputs["bn_g2"],
        inputs["bn_b2"], inputs["bn_g3"], inputs["bn_b3"], inputs["bn_g4"],
        inputs["bn_b4"], inputs["fc_w1"], inputs["fc_b1"], inputs["fc_w2"],
        inputs["fc_b2"])
    shards = _shard_x(inputs["x"])
    return [{**consts, "xn": xn} for xn in shards]


def kernel(**inputs) -> np.ndarray:
    from concourse.bass_utils import run_bass_kernel_spmd
    nc = get_nc()
    in_maps = make_in_maps(inputs)
    res = run_bass_kernel_spmd(nc, in_maps, list(range(N_CORES)))
    return np.concatenate([res.results[c]["y"] for c in range(N_CORES)], axis=0)
